# revision 1
# baseline (speedup 1.0000x reference)
"""Trainium2 Bass kernel for nn_CLLayer (SimCLR-style contrastive loss).

Math (reference, tau=0.5):
    h1 = elu(z1 @ W1.T + b1) @ W2.T + b2 ; h2 likewise
    n1, n2 = row-normalized h1, h2
    l1_i = log(sum_j exp(2*n1_i.n1_j) + sum_j exp(2*n1_i.n2_j) - e^2) - 2*n1_i.n2_i
    l2_i = log(sum_j exp(2*n2_i.n2_j) + sum_j exp(2*n2_j.n1_i... ) - e^2) - 2*...
    out = 0.5*(l1+l2)

Sharding: row-parallel over N=8192 (1024 rows/core, 8 cores).
Each core: projects its row block (bf16 matmuls), normalizes, AllGathers
normalized embeddings (bf16), computes its row-strip of the three distinct
similarity products (S12, S22, S11), exp+row-sums on the fly, column-sums of
exp(2*S12) via a ReduceScatter (between2 = between.T so l2's "between" row
sums are column sums of S12's exp).  Only 3 of 4 N^2*D products are needed.

Host-side prep: transposes z blocks / weights to K-major (PE wants K on
partitions), casts matmul operands to bf16, and folds the ELU "-1" into an
adjusted fc2 bias (b2' = b2 - fc2_w.sum(1)) so ELU is computed as
relu(x) + exp(min(x,0)) without the subtract (device ELU' = elu + 1).
"""

import math
import os
from functools import lru_cache

import ml_dtypes
import numpy as np

import concourse.bacc as bacc
import concourse.bass as bass
import concourse.mybir as mybir
import concourse.tile as tile
from concourse.bass_utils import run_bass_kernel_spmd

N, D = 8192, 1024
NCORES = 8
BLK = N // NCORES  # 1024
P = 128
KO = D // P  # 8 k-tiles
NT = BLK // P  # 8 i-tiles per core
JC = N // 512  # 16 j-chunks of 512
E2 = float(np.exp(2.0))  # exp(1/tau), tau=0.5
BF = mybir.dt.bfloat16
F32 = mybir.dt.float32
AF = mybir.ActivationFunctionType
ALU = mybir.AluOpType


def _build():
    nc = bacc.Bacc("TRN2", target_bir_lowering=False, debug=False, num_devices=NCORES)

    z1t = nc.dram_tensor("z1t", [D, BLK], BF, kind="ExternalInput")
    z2t = nc.dram_tensor("z2t", [D, BLK], BF, kind="ExternalInput")
    w1t = nc.dram_tensor("w1t", [D, D], BF, kind="ExternalInput")
    w2t = nc.dram_tensor("w2t", [D, D], BF, kind="ExternalInput")
    b1 = nc.dram_tensor("b1", [D], F32, kind="ExternalInput")
    b2p = nc.dram_tensor("b2p", [D], F32, kind="ExternalInput")
    out = nc.dram_tensor("out", [BLK], F32, kind="ExternalOutput")

    kp = lambda ap: ap.rearrange("(ko ki) x -> ki ko x", ki=P)  # K-major -> [128, KO, x]
    pt = lambda ap: ap.rearrange("(t p) -> p t", p=P)  # [1024] -> [128, 8]
    JP = JC // 2  # 8 j-chunk-pairs of 1024

    with tile.TileContext(nc) as tc:
        with (
            tc.tile_pool(name="consts", bufs=1) as consts,
            tc.tile_pool(name="mats", bufs=1) as mats,
            tc.tile_pool(name="strip", bufs=1) as strip,
            tc.tile_pool(name="scratch", bufs=2) as scratch,
            tc.tile_pool(name="rhs", bufs=3) as rhsp,
            tc.tile_pool(name="expp", bufs=2) as expp,
            tc.tile_pool(name="small", bufs=1) as small,
            tc.tile_pool(name="psA", bufs=3, space="PSUM") as psA,
            tc.tile_pool(name="psB", bufs=2, space="PSUM") as psB,
            tc.tile_pool(name="dram", bufs=1, space="DRAM") as dram,
        ):
            # ---------------- constants ----------------
            w1_sb = consts.tile([P, KO, D], BF)
            w2_sb = consts.tile([P, KO, D], BF)
            nc.sync.dma_start(w1_sb[:], kp(w1t[:]))
            nc.sync.dma_start(w2_sb[:], kp(w2t[:]))
            b1_sb = consts.tile([P, KO], F32)
            b2_sb = consts.tile([P, KO], F32)
            nc.sync.dma_start(b1_sb[:], pt(b1[:]))
            nc.sync.dma_start(b2_sb[:], pt(b2p[:]))
            ones_bf = consts.tile([P, 1], BF)
            ones_f = consts.tile([P, 1], F32)
            nc.vector.memset(ones_bf[:], 1.0)
            nc.vector.memset(ones_f[:], 1.0)

            z_sb = mats.tile([P, KO, BLK], BF, tag="zt")
            n1_sb = mats.tile([P, KO, BLK], BF, tag="n1")
            n2_sb = mats.tile([P, KO, BLK], BF, tag="n2")

            ag1_in = dram.tile([D, BLK], BF)
            ag2_in = dram.tile([D, BLK], BF)
            ag1_out = dram.tile([NCORES, D, BLK], BF, addr_space="Shared")
            ag2_out = dram.tile([NCORES, D, BLK], BF, addr_space="Shared")
            rs_in = dram.tile([N], F32)
            rs_out = dram.tile([BLK], F32)
            rn_dram = dram.tile([2, BLK], BF)
            p_dram = dram.tile([BLK], F32)

            # ------------ projection + normalize (into n_sb), per tensor ------------
            def project(z_at, elu_sb, n_sb, rn_slot):
                # layer 1: a1T[o, i] = W1T.T @ zT (K=d);
                # elu+1 = relu(a+b1) + min(exp(a+b1), 1)
                for ot in range(KO):
                    ps = psA.tile([P, 1024], F32, tag="ps_big")
                    for ch in range(2):
                        sl = bass.ts(ch, 512)
                        for kt in range(KO):
                            nc.tensor.matmul(
                                ps[:, sl],
                                w1_sb[:, kt, bass.ts(ot, P)],
                                z_at(kt, ch),
                                start=(kt == 0),
                                stop=(kt == KO - 1),
                            )
                    bcol = b1_sb[:, ot : ot + 1]
                    e_t = scratch.tile([P, 1024], F32, tag="e_t")
                    r_t = scratch.tile([P, 1024], F32, tag="r_t")
                    nc.scalar.activation(e_t[:], ps[:], AF.Exp, bias=bcol)
                    nc.scalar.activation(r_t[:], ps[:], AF.Relu, bias=bcol)
                    nc.vector.tensor_scalar(e_t[:], e_t[:], 1.0, None, ALU.min)
                    nc.vector.tensor_tensor(elu_sb[:, ot, :], e_t[:], r_t[:], ALU.add)
                # layer 2 -> n_sb (holds hT until scaled in place)
                for ot in range(KO):
                    ps = psA.tile([P, 1024], F32, tag="ps_big")
                    for ch in range(2):
                        sl = bass.ts(ch, 512)
                        for kt in range(KO):
                            nc.tensor.matmul(
                                ps[:, sl],
                                w2_sb[:, kt, bass.ts(ot, P)],
                                elu_sb[:, kt, bass.ds(ch * 512, 512)],
                                start=(kt == 0),
                                stop=(kt == KO - 1),
                            )
                    nc.vector.tensor_scalar(
                        n_sb[:, ot, :], ps[:], b2_sb[:, ot : ot + 1], None, ALU.add
                    )
                # sumsq over d (partitions) via ones-matmul on Square(h)
                ssps = [psB.tile([1, 512], F32, name=f"ssps{_c}", tag="ps_small") for _c in range(2)]
                for kt in range(KO):
                    sq = scratch.tile([P, BLK], BF, tag="sq")
                    nc.scalar.activation(sq[:], n_sb[:, kt, :], AF.Square)
                    for ch in range(2):
                        nc.tensor.matmul(
                            ssps[ch][:],
                            ones_bf[:],
                            sq[:, bass.ts(ch, 512)],
                            start=(kt == 0),
                            stop=(kt == KO - 1),
                        )
                # rn = 1/||h|| per column, one Newton step on top of 1/sqrt
                rn_bf = small.tile([1, BLK], BF, tag="rn_bf")
                for ch in range(2):
                    sl = bass.ts(ch, 512)
                    ssq_c = small.tile([1, 512], F32, tag="ssq_c", name=f"ssq_c{ch}")
                    nrm_c = small.tile([1, 512], F32, tag="nrm_c", name=f"nrm_c{ch}")
                    y_c = small.tile([1, 512], F32, tag="y_c", name=f"y_c{ch}")
                    t1_c = small.tile([1, 512], F32, tag="t1_c", name=f"t1_c{ch}")
                    nc.vector.tensor_copy(ssq_c[:], ssps[ch][:])
                    nc.scalar.activation(nrm_c[:], ssps[ch][:], AF.Sqrt)
                    nc.vector.reciprocal(y_c[:], nrm_c[:])
                    nc.vector.tensor_tensor(t1_c[:], y_c[:], y_c[:], ALU.mult)
                    nc.vector.tensor_tensor(t1_c[:], t1_c[:], ssq_c[:], ALU.mult)
                    nc.vector.tensor_scalar(t1_c[:], t1_c[:], -0.5, 1.5, ALU.mult, ALU.add)
                    nc.vector.tensor_tensor(t1_c[:], y_c[:], t1_c[:], ALU.mult)
                    nc.vector.tensor_copy(rn_bf[:, sl], t1_c[:])
                nc.scalar.dma_start(rn_dram[rn_slot : rn_slot + 1, :], rn_bf[:])
                rn_bc = scratch.tile([P, BLK], BF, tag="rnbc", bufs=1)
                nc.scalar.dma_start(rn_bc[:], rn_dram[rn_slot : rn_slot + 1, :].to_broadcast((P, BLK)))
                for kt in range(KO):
                    nc.vector.tensor_tensor(n_sb[:, kt, :], n_sb[:, kt, :], rn_bc[:], ALU.mult)

            rg = [list(range(NCORES))]
            # z1 into its slot; z2 into the (idle until pass A) rhs-pool slots so
            # both projections can interleave on the PE.
            nc.sync.dma_start(z_sb[:], kp(z1t[:]))
            z2a = rhsp.tile([P, KO, 512], BF, tag="rhs", name="z2a")
            z2b = rhsp.tile([P, KO, 512], BF, tag="rhs", name="z2b")
            nc.sync.dma_start(z2a[:], kp(z2t[:, 0:512]))
            nc.sync.dma_start(z2b[:], kp(z2t[:, 512:1024]))
            elu1 = mats.tile([P, KO, BLK], BF, tag="elu")
            project(lambda kt, ch: z_sb[:, kt, bass.ds(ch * 512, 512)], elu1, n1_sb, 0)
            nc.scalar.dma_start(kp(ag1_in[:]), n1_sb[:])
            nc.gpsimd.collective_compute(
                "AllGather", ALU.bypass, replica_groups=rg,
                ins=[ag1_in[:].opt()], outs=[ag1_out[:].opt()],
            )
            # elu2 reuses the z1 slot (z1 dead after its layer 1)
            elu2 = mats.tile([P, KO, BLK], BF, tag="zt", name="elu2")
            project(lambda kt, ch: (z2a if ch == 0 else z2b)[:, kt, :], elu2, n2_sb, 1)
            nc.scalar.dma_start(kp(ag2_in[:]), n2_sb[:])
            nc.gpsimd.collective_compute(
                "AllGather", ALU.bypass, replica_groups=rg,
                ins=[ag2_in[:].opt()], outs=[ag2_out[:].opt()],
            )

            # ---------------- p_i = n1_i . n2_i (local diag of S12) ----------------
            pps = [psB.tile([1, 512], F32, name=f"pps{_c}", tag="ps_small") for _c in range(2)]
            for kt in range(KO):
                q = scratch.tile([P, BLK], BF, tag="sq")
                nc.vector.tensor_tensor(q[:], n1_sb[:, kt, :], n2_sb[:, kt, :], ALU.mult)
                for ch in range(2):
                    nc.tensor.matmul(
                        pps[ch][:],
                        ones_bf[:],
                        q[:, bass.ts(ch, 512)],
                        start=(kt == 0),
                        stop=(kt == KO - 1),
                    )
            for ch in range(2):
                p_c = small.tile([1, 512], F32, tag="ssq_c", name=f"p_c{ch}")
                nc.vector.tensor_copy(p_c[:], pps[ch][:])
                nc.gpsimd.dma_start(p_dram[ch * 512 : (ch + 1) * 512], p_c[:])

            # rowsum partials, one column per j-chunk-pair
            r11p = strip.tile([P, NT, JP], F32)
            r12p = strip.tile([P, NT, JP], F32)
            r22p = strip.tile([P, NT, JP], F32)
            cs = strip.tile([P, N], F32)  # exp(2*S12) partial column sums

            def rhs_pair(ag, jp):
                a = rhsp.tile([P, KO, 512], BF, tag="rhs", name=f"rhs_a{jp}")
                b = rhsp.tile([P, KO, 512], BF, tag="rhs", name=f"rhs_b{jp}")
                blk = kp(ag[jp])
                nc.sync.dma_start(a[:], blk[:, :, 0:512])
                nc.sync.dma_start(b[:], blk[:, :, 512:1024])
                return a, b

            def sim_iter(lhs, tt, rta, rtb, accum, s12_jp=None):
                ps = psA.tile([P, 1024], F32, tag="ps_big", name="ps_sim")
                for ch, rt in ((0, rta), (1, rtb)):
                    sl = bass.ts(ch, 512)
                    for kt in range(KO):
                        nc.tensor.matmul(
                            ps[:, sl],
                            lhs[:, kt, bass.ts(tt, P)],
                            rt[:, kt, :],
                            start=(kt == 0),
                            stop=(kt == KO - 1),
                        )
                ex = expp.tile([P, 1024], F32, tag="ex")
                nc.scalar.activation(ex[:], ps[:], AF.Exp, scale=2.0, accum_out=accum)
                if s12_jp is not None:
                    csl = cs[:, bass.ds(s12_jp * 1024, 1024)]
                    nc.vector.tensor_tensor(csl, csl, ex[:], ALU.add)

            # ---- pass A: S11 (lhs n1, rhs gathered n1) ----
            for jp in range(JP):
                rta, rtb = rhs_pair(ag1_out, jp)
                for tt in range(NT):
                    sim_iter(n1_sb, tt, rta, rtb, r11p[:, tt, jp : jp + 1])

            # ---- pass B1: S12 (lhs n1, rhs gathered n2) + incremental colsums ----
            nc.vector.memset(cs[:], 0.0)
            for jp in range(JP):
                rta, rtb = rhs_pair(ag2_out, jp)
                for tt in range(NT):
                    sim_iter(n1_sb, tt, rta, rtb, r12p[:, tt, jp : jp + 1], s12_jp=jp)
                # this 1024-wide slice of cs is complete -> reduce over partitions
                for h in range(2):
                    cp = psB.tile([1, 512], F32, tag="ps_small", name=f"cp{jp}_{h}")
                    nc.tensor.matmul(
                        cp[:], ones_f[:], cs[:, bass.ds(jp * 1024 + h * 512, 512)],
                        start=True, stop=True,
                    )
                    cst = scratch.tile([1, 512], F32, tag="cst", bufs=2, name=f"cst{jp}_{h}")
                    nc.vector.tensor_copy(cst[:], cp[:])
                    nc.gpsimd.dma_start(
                        rs_in[(jp * 2 + h) * 512 : (jp * 2 + h + 1) * 512], cst[:]
                    )
            nc.gpsimd.collective_compute(
                "ReduceScatter", ALU.add, replica_groups=rg,
                ins=[rs_in[:].opt()], outs=[rs_out[:].opt()],
            )

            # ---- pass B2: S22 (lhs n2, rhs gathered n2); RS overlaps this ----
            for jp in range(JP):
                rta, rtb = rhs_pair(ag2_out, jp)
                for tt in range(NT):
                    sim_iter(n2_sb, tt, rta, rtb, r22p[:, tt, jp : jp + 1])

            # ---------------- final loss ----------------
            r11 = small.tile([P, NT], F32, tag="r11")
            r12 = small.tile([P, NT], F32, tag="r12")
            r22 = small.tile([P, NT], F32, tag="r22")
            nc.vector.reduce_sum(r11[:], r11p[:], axis=mybir.AxisListType.X)
            nc.vector.reduce_sum(r12[:], r12p[:], axis=mybir.AxisListType.X)
            nc.vector.reduce_sum(r22[:], r22p[:], axis=mybir.AxisListType.X)
            c12 = small.tile([P, NT], F32, tag="c12")
            nc.sync.dma_start(c12[:], pt(rs_out[:]))
            p2 = small.tile([P, NT], F32, tag="p2")
            nc.sync.dma_start(p2[:], pt(p_dram[:]))

            d1 = small.tile([P, NT], F32, tag="d1")
            d2 = small.tile([P, NT], F32, tag="d2")
            nc.vector.tensor_tensor(d1[:], r11[:], r12[:], ALU.add)
            nc.vector.tensor_scalar(d1[:], d1[:], -E2, None, ALU.add)
            nc.vector.tensor_tensor(d2[:], r22[:], c12[:], ALU.add)
            nc.vector.tensor_scalar(d2[:], d2[:], -E2, None, ALU.add)
            l1 = small.tile([P, NT], F32, tag="l1")
            l2 = small.tile([P, NT], F32, tag="l2")
            nc.scalar.activation(l1[:], d1[:], AF.Ln)
            nc.scalar.activation(l2[:], d2[:], AF.Ln)
            loss = small.tile([P, NT], F32, tag="loss")
            nc.vector.tensor_tensor(loss[:], l1[:], l2[:], ALU.add)
            nc.vector.tensor_scalar(loss[:], loss[:], 0.5, None, ALU.mult)
            pm = small.tile([P, NT], F32, tag="pm")
            nc.vector.tensor_scalar(pm[:], p2[:], -2.0, None, ALU.mult)
            nc.vector.tensor_tensor(loss[:], loss[:], pm[:], ALU.add)
            nc.sync.dma_start(pt(out[:]), loss[:])

    nc.finalize()
    return nc


@lru_cache(maxsize=1)
def _built():
    return _build()


def _prep_inputs(z1, z2, fc1_w, fc1_b, fc2_w, fc2_b):
    bf = ml_dtypes.bfloat16
    w1t = np.ascontiguousarray(np.asarray(fc1_w, np.float32).T).astype(bf)
    w2t = np.ascontiguousarray(np.asarray(fc2_w, np.float32).T).astype(bf)
    b1 = np.asarray(fc1_b, np.float32)
    b2p = (np.asarray(fc2_b, np.float32) - np.asarray(fc2_w, np.float32).sum(axis=1)).astype(
        np.float32
    )
    in_maps = []
    for c in range(NCORES):
        sl = slice(c * BLK, (c + 1) * BLK)
        in_maps.append(
            {
                "z1t": np.ascontiguousarray(np.asarray(z1[sl], np.float32).T).astype(bf),
                "z2t": np.ascontiguousarray(np.asarray(z2[sl], np.float32).T).astype(bf),
                "w1t": w1t,
                "w2t": w2t,
                "b1": b1,
                "b2p": b2p,
            }
        )
    return in_maps


def _install_ntff_shim():
    """Register the axon NTFF profile hook (antenv.axon_hooks is absent in
    this image; rebuild it from trn_agent_boot's ctypes recipe)."""
    import sys
    import types

    if "antenv.axon_hooks" in sys.modules:
        return True
    try:
        import antenv
        from trn_agent_boot.trn_boot import _ntff_profile_via_ctypes

        hook = _ntff_profile_via_ctypes("/opt/axon/libaxon_pjrt.so")
        if hook is None:
            return False
        m = types.ModuleType("antenv.axon_hooks")
        m._hook = hook
        m.get_axon_ntff_profile_hook = lambda: m._hook
        m.set_axon_ntff_profile_hook = lambda h: setattr(m, "_hook", h)
        sys.modules["antenv.axon_hooks"] = m
        antenv.axon_hooks = m
        # artifact upload needs egress; neuter it for local profiling
        import concourse.bass_utils as _bu

        _bu.upload_artifacts = lambda tmpdir: f"file://{tmpdir}"
        return True
    except Exception as e:
        print(f"ntff shim unavailable: {e!r}")
        return False


def _run(in_maps, trace=False):
    nc = _built()
    if trace and not _install_ntff_shim():
        trace = False
    last = None
    for attempt in range(3):
        try:
            res = run_bass_kernel_spmd(nc, in_maps, list(range(NCORES)), trace=trace)
            if all(np.isfinite(res.results[c]["out"]).all() for c in range(NCORES)):
                return res
            print("nonfinite output, retrying")
        except Exception as e:  # device occasionally wedged from a prior process
            last = e
            if "UNRECOVERABLE" not in str(e) and "UNAVAILABLE" not in str(e):
                raise
            print(f"device error (attempt {attempt}): retrying")
    if last is not None:
        raise last
    return res


def kernel(z1, z2, fc1_w, fc1_b, fc2_w, fc2_b):
    in_maps = _prep_inputs(z1, z2, fc1_w, fc1_b, fc2_w, fc2_b)
    res = _run(in_maps, trace=os.environ.get("KERNEL_TRACE", "") == "1")
    if res.exec_time_ns is not None:
        print(f"HW exec time: {res.exec_time_ns} ns")
    out = np.concatenate([res.results[c]["out"] for c in range(NCORES)])
    return out.astype(np.float32)



# revision 10
# speedup vs baseline: 1.7794x; 1.7794x over previous
"""Trainium2 Bass kernel for nn_CLLayer (SimCLR-style contrastive loss).

Math (reference, tau=0.5):
    h1 = elu(z1 @ W1.T + b1) @ W2.T + b2 ; h2 likewise
    n1, n2 = row-normalized h1, h2
    l1_i = log(sum_j exp(2*n1_i.n1_j) + sum_j exp(2*n1_i.n2_j) - e^2) - 2*n1_i.n2_i
    l2_i = log(sum_j exp(2*n2_i.n2_j) + sum_j exp(2*n1_j.n2_i) - e^2) - 2*n1_i.n2_i
    out = 0.5*(l1+l2)

Strategy (row-parallel over N=8192, 1024 rows/core, 8 cores):
 - FP8(e4m3) DoubleRow matmuls everywhere (projection + similarity): K=256
   per instruction at 0.5 cycles/row -> ~4x bf16 MAC rate.
 - S11/S22 are symmetric: each core computes, for EVERY absolute strip j
   (uniform SPMD addresses), only quadrant Q01 (rows 0:512 x cols 512:1024)
   at full weight plus the two diagonal quadrants Q00/Q11 scaled by 1/2
   (exp bias = ln 1/2). The transposed half of each pair-block arrives as
   column sums from the partner core via ReduceScatter. Work = 1/2 of the
   full strip per sym matrix. S12 is not symmetric -> full strips, with
   column sums giving l2's "between" row sums.
 - exp tiles are written in fp8; column sums are ones-vector DoubleRow
   matmuls accumulated in PSUM (no vector-engine traffic). Row sums ride
   free on the activation accum_out (f32, pre-cast).
 - Normalized embeddings are scaled x16 and cast fp8 before a fp8
   AllGather; sim psums are descaled in the exp (scale=2/256).
 - pos diag p_i = n1_i.n2_i comes from a separate bf16 path (h1*h2 ones
   reduction times f32 1/norms) for accuracy.

Host-side prep: K-major transposes, fp8 casts, and b2' = b2 - sum_k W2_f8
so ELU is computed as relu(x) + min(exp(x),1) (device ELU' = elu + 1).
"""

import math
import os
from functools import lru_cache

import ml_dtypes
import numpy as np

import concourse.bacc as bacc
import concourse.bass as bass
import concourse.mybir as mybir
import concourse.tile as tile
from concourse.bass_utils import run_bass_kernel_spmd

N, D = 8192, 1024
NCORES = 8
BLK = N // NCORES  # 1024
P = 128
KO = D // P  # 8 k-tiles
KO2 = KO // 2  # 4 double-row k-pairs
NT = BLK // P  # 8 i-tiles per core
E2 = float(np.exp(2.0))  # exp(1/tau), tau=0.5
SC = 2.0 / 256.0  # exp scale: tau and the 16x16 fp8 prescale
LN_HALF = float(math.log(0.5))
BF = mybir.dt.bfloat16
F32 = mybir.dt.float32
F8 = mybir.dt.float8e4
AF = mybir.ActivationFunctionType
ALU = mybir.AluOpType
DR = mybir.MatmulPerfMode.DoubleRow


def _build():
    nc = bacc.Bacc("TRN2", target_bir_lowering=False, debug=False, num_devices=NCORES)

    z1t = nc.dram_tensor("z1t", [D, BLK], F8, kind="ExternalInput")
    z2t = nc.dram_tensor("z2t", [D, BLK], F8, kind="ExternalInput")
    w1t = nc.dram_tensor("w1t", [D, D], F8, kind="ExternalInput")
    w2t = nc.dram_tensor("w2t", [D, D], F8, kind="ExternalInput")
    b1 = nc.dram_tensor("b1", [D], F32, kind="ExternalInput")
    b2p = nc.dram_tensor("b2p", [D], F32, kind="ExternalInput")
    out = nc.dram_tensor("out", [BLK], F32, kind="ExternalOutput")

    kp = lambda ap: ap.rearrange("(ko ki) x -> ki ko x", ki=P)  # K-major -> [128, KO, x]
    pt = lambda ap: ap.rearrange("(t p) -> p t", p=P)  # [1024] -> [128, 8]

    with tile.TileContext(nc) as tc:
        with (
            tc.tile_pool(name="consts", bufs=1) as consts,
            tc.tile_pool(name="mats", bufs=1) as mats,
            tc.tile_pool(name="strip", bufs=1) as strip,
            tc.tile_pool(name="scratch", bufs=2) as scratch,
            tc.tile_pool(name="rhs", bufs=3) as rhsp,
            tc.tile_pool(name="expp", bufs=6) as expp,
            tc.tile_pool(name="small", bufs=1) as small,
            tc.tile_pool(name="psA", bufs=3, space="PSUM") as psA,
            tc.tile_pool(name="psB", bufs=2, space="PSUM") as psB,
            tc.tile_pool(name="dram", bufs=1, space="DRAM") as dram,
        ):
            # ---------------- constants ----------------
            w1_sb = consts.tile([P, KO, D], F8)
            w2_sb = consts.tile([P, KO, D], F8)
            nc.sync.dma_start(w1_sb[:], kp(w1t[:]))
            nc.sync.dma_start(w2_sb[:], kp(w2t[:]))
            b1_sb = consts.tile([P, KO], F32)
            b2_sb = consts.tile([P, KO], F32)
            nc.sync.dma_start(b1_sb[:], pt(b1[:]))
            nc.sync.dma_start(b2_sb[:], pt(b2p[:]))
            ones8 = consts.tile([P, 2, 16], F8)
            ones_bf = consts.tile([P, 1], BF)
            lnhalf = consts.tile([P, 1], F32)
            nc.vector.memset(ones8[:], 1.0)
            nc.vector.memset(ones_bf[:], 1.0)
            nc.vector.memset(lnhalf[:], LN_HALF)

            z1_sb = mats.tile([P, KO, BLK], F8, tag="z1")
            z2_sb = mats.tile([P, KO, BLK], F8, tag="z2")
            nc.sync.dma_start(z1_sb[:], kp(z1t[:]))
            nc.sync.dma_start(z2_sb[:], kp(z2t[:]))
            h1_sb = mats.tile([P, KO, BLK], BF, tag="h1")
            h2_sb = mats.tile([P, KO, BLK], BF, tag="h2")
            n1_f8 = mats.tile([P, KO, BLK], F8, tag="n1")
            n2_f8 = mats.tile([P, KO, BLK], F8, tag="n2")

            ag1_in = dram.tile([D, BLK], F8)
            ag2_in = dram.tile([D, BLK], F8)
            ag1_out = dram.tile([NCORES, D, BLK], F8, addr_space="Shared")
            ag2_out = dram.tile([NCORES, D, BLK], F8, addr_space="Shared")
            rs_in = dram.tile([NCORES, 3, BLK], F32)
            rs_out = dram.tile([3, BLK], F32)
            rn_dram = dram.tile([2, BLK], BF)
            p_dram = dram.tile([BLK], F32)

            # rowsum partials: one column per (strip, quadrant-group)
            D1p = strip.tile([P, NT, 3 * NCORES], F32)  # S11: 2/strip, S12: 1/strip
            D2p = strip.tile([P, NT, 2 * NCORES], F32)  # S22: 2/strip
            nc.vector.memset(D1p[:], 0.0)
            nc.vector.memset(D2p[:], 0.0)

            rn_f = [
                small.tile([1, BLK], F32, tag=f"rn_f{i}", name=f"rn_f{i}")
                for i in range(2)
            ]

            def dr_group(ps_sl, lhs, tt, rt, col):
                """K=1024 fp8 DoubleRow accumulation: out [128, 512]."""
                for k2 in range(KO2):
                    nc.tensor.matmul(
                        ps_sl,
                        lhs[:, 2 * k2 : 2 * k2 + 2, bass.ts(tt, P)],
                        rt[:, 2 * k2 : 2 * k2 + 2, bass.ds(col, 512)],
                        start=(k2 == 0),
                        stop=(k2 == KO2 - 1),
                        perf_mode=DR,
                    )

            # ------------ projection + normalize, per tensor ------------
            def project(z_sb, elu_sb, h_sb, rn_slot):
                # layer 1: elu+1 = relu(a+b1) + min(exp(a+b1), 1)
                for ot in range(KO):
                    ps = psA.tile([P, 1024], F32, tag="ps_big")
                    for ch in range(2):
                        dr_group(ps[:, bass.ts(ch, 512)], w1_sb, ot, z_sb, ch * 512)
                    bcol = b1_sb[:, ot : ot + 1]
                    e_t = scratch.tile([P, 1024], F32, tag="e_t")
                    r_t = scratch.tile([P, 1024], F32, tag="r_t")
                    nc.scalar.activation(e_t[:], ps[:], AF.Exp, bias=bcol)
                    nc.vector.tensor_scalar(r_t[:], ps[:], bcol, 0.0, ALU.add, ALU.max)
                    nc.vector.tensor_scalar(e_t[:], e_t[:], 1.0, None, ALU.min)
                    nc.vector.tensor_tensor(elu_sb[:, ot, :], e_t[:], r_t[:], ALU.add)
                # layer 2 -> h (bf16)
                for ot in range(KO):
                    ps = psA.tile([P, 1024], F32, tag="ps_big")
                    for ch in range(2):
                        dr_group(ps[:, bass.ts(ch, 512)], w2_sb, ot, elu_sb, ch * 512)
                    nc.vector.tensor_scalar(
                        h_sb[:, ot, :], ps[:], b2_sb[:, ot : ot + 1], None, ALU.add
                    )
                # sumsq over d via bf16 ones-matmul on h*h
                ssps = [
                    psB.tile([16, 512], F32, name=f"ssps{c}", tag="ps_wide")
                    for c in range(2)
                ]
                for kt in range(KO):
                    sq = scratch.tile([P, BLK], BF, tag="sq")
                    nc.vector.tensor_tensor(sq[:], h_sb[:, kt, :], h_sb[:, kt, :], ALU.mult)
                    for ch in range(2):
                        nc.tensor.matmul(
                            ssps[ch][0:1, :],
                            ones_bf[:],
                            sq[:, bass.ts(ch, 512)],
                            start=(kt == 0),
                            stop=(kt == KO - 1),
                        )
                # rn = 1/||h||: one Newton step on 1/sqrt; keep f32 (for p) and
                # 16x bf16 broadcast (for fp8 n)
                rn16_bf = small.tile([1, BLK], BF, tag="rn16_bf")
                for ch in range(2):
                    sl = bass.ts(ch, 512)
                    ssq_c = small.tile([1, 512], F32, tag="ssq_c", name=f"ssq_c{ch}")
                    nrm_c = small.tile([1, 512], F32, tag="nrm_c", name=f"nrm_c{ch}")
                    y_c = small.tile([1, 512], F32, tag="y_c", name=f"y_c{ch}")
                    t1_c = small.tile([1, 512], F32, tag="t1_c", name=f"t1_c{ch}")
                    nc.vector.tensor_copy(ssq_c[:], ssps[ch][0:1, :])
                    nc.scalar.activation(nrm_c[:], ssps[ch][0:1, :], AF.Sqrt)
                    nc.vector.reciprocal(y_c[:], nrm_c[:])
                    nc.vector.tensor_tensor(t1_c[:], y_c[:], y_c[:], ALU.mult)
                    nc.vector.tensor_tensor(t1_c[:], t1_c[:], ssq_c[:], ALU.mult)
                    nc.vector.tensor_scalar(t1_c[:], t1_c[:], -0.5, 1.5, ALU.mult, ALU.add)
                    nc.vector.tensor_tensor(t1_c[:], y_c[:], t1_c[:], ALU.mult)
                    nc.vector.tensor_copy(rn_f[rn_slot][:, sl], t1_c[:])
                    nc.vector.tensor_scalar(t1_c[:], t1_c[:], 16.0, None, ALU.mult)
                    nc.vector.tensor_copy(rn16_bf[:, sl], t1_c[:])
                nc.scalar.dma_start(rn_dram[rn_slot : rn_slot + 1, :], rn16_bf[:])
                rn_bc = scratch.tile([P, BLK], BF, tag="rnbc", bufs=1)
                nc.scalar.dma_start(
                    rn_bc[:], rn_dram[rn_slot : rn_slot + 1, :].to_broadcast((P, BLK))
                )
                n_f8 = n1_f8 if rn_slot == 0 else n2_f8
                for kt in range(KO):
                    nc.vector.tensor_tensor(
                        n_f8[:, kt, :], h_sb[:, kt, :], rn_bc[:], ALU.mult
                    )

            rg = [list(range(NCORES))]
            elu1 = mats.tile([P, KO, BLK], F8, tag="elu", name="elu1")
            project(z1_sb, elu1, h1_sb, 0)
            nc.scalar.dma_start(kp(ag1_in[:]), n1_f8[:])
            nc.gpsimd.collective_compute(
                "AllGather", ALU.bypass, replica_groups=rg,
                ins=[ag1_in[:].opt()], outs=[ag1_out[:].opt()],
            )
            # z1 is dead after proj1 layer 1 -> reuse its slot for elu2
            elu2 = mats.tile([P, KO, BLK], F8, tag="z1", name="elu2")
            project(z2_sb, elu2, h2_sb, 1)
            nc.scalar.dma_start(kp(ag2_in[:]), n2_f8[:])
            nc.gpsimd.collective_compute(
                "AllGather", ALU.bypass, replica_groups=rg,
                ins=[ag2_in[:].opt()], outs=[ag2_out[:].opt()],
            )

            # ------ p_i = n1_i . n2_i via bf16 h1*h2 and f32 1/norms ------
            pps = [
                psB.tile([16, 512], F32, name=f"pps{c}", tag="ps_wide") for c in range(2)
            ]
            for kt in range(KO):
                hq = scratch.tile([P, BLK], BF, tag="sq", name=f"hq{kt}")
                nc.vector.tensor_tensor(hq[:], h1_sb[:, kt, :], h2_sb[:, kt, :], ALU.mult)
                for ch in range(2):
                    nc.tensor.matmul(
                        pps[ch][0:1, :],
                        ones_bf[:],
                        hq[:, bass.ts(ch, 512)],
                        start=(kt == 0),
                        stop=(kt == KO - 1),
                    )
            for ch in range(2):
                sl = bass.ts(ch, 512)
                p_c = small.tile([1, 512], F32, tag="ssq_c", name=f"p_c{ch}")
                nc.vector.tensor_copy(p_c[:], pps[ch][0:1, :])
                nc.vector.tensor_tensor(p_c[:], p_c[:], rn_f[0][:, sl], ALU.mult)
                nc.vector.tensor_tensor(p_c[:], p_c[:], rn_f[1][:, sl], ALU.mult)
                nc.gpsimd.dma_start(p_dram[ch * 512 : (ch + 1) * 512], p_c[:])

            # ---------------- similarity passes ----------------
            def rhs_load(ag, j, nm):
                t = rhsp.tile([P, KO, BLK], F8, tag="rhs", name=nm)
                blk = kp(ag[j])
                nc.sync.dma_start(t[:, :, 0:512], blk[:, :, 0:512])
                nc.sync.dma_start(t[:, :, 512:1024], blk[:, :, 512:1024])
                return t

            # Colsums are deferred by one strip: the ones-matmuls read scalar
            # engine exp outputs, so emitting them after the NEXT strip's sim
            # matmuls keeps the PE from stalling on the Act engine.
            pending = []

            def flush_pending():
                while pending:
                    pending.pop(0)()

            def colsum(j, rs_slot, h, ex_tiles, nm):
                """PSUM-accumulated fp8 ones DoubleRow colsum -> rs_in[j, slot, h]."""

                def emit():
                    cps = psB.tile([16, 512], F32, tag="ps_wide", name=f"cps{nm}")
                    for i, (ext, esl) in enumerate(ex_tiles):
                        nc.tensor.matmul(
                            cps[:],
                            ones8[:],
                            ext[:, :, esl] if esl is not None else ext[:],
                            start=(i == 0),
                            stop=(i == len(ex_tiles) - 1),
                            perf_mode=DR,
                        )
                    cst = scratch.tile([1, 512], F32, tag="cst", bufs=2, name=f"cst{nm}")
                    nc.vector.tensor_copy(cst[:], cps[0:1, :])
                    nc.gpsimd.dma_start(
                        rs_in[j : j + 1, rs_slot : rs_slot + 1, bass.ds(h * 512, 512)],
                        cst[:],
                    )

                pending.append(emit)

            def sym_strip(lhs, ag, j, Dp, slot0, rs_slot, nm):
                """Symmetric-half strip: Q01 full + Q00/Q11 at exp weight 1/2."""
                rt = rhs_load(ag, j, f"r{nm}")
                # column half 0: Q00 (tts 0-3, cols 0:512, ln(1/2) bias)
                exq = [
                    expp.tile([P, 2, 512], F8, tag="exq", bufs=12, name=f"x{nm}_{i}")
                    for i in range(2)
                ]
                for tt in range(4):
                    ps = psA.tile([P, 1024], F32, tag="ps_big", name=f"p0{nm}_{tt}")
                    dr_group(ps[:, 0:512], lhs, tt, rt, 0)
                    if tt == 0:
                        flush_pending()
                    nc.scalar.activation(
                        exq[tt // 2][:, tt % 2, :], ps[:, 0:512], AF.Exp,
                        scale=SC, bias=lnhalf[:],
                        accum_out=Dp[:, tt, slot0 + 1 : slot0 + 2],
                    )
                # column half 1: Q01 (tts 0-3, full) + Q11 (tts 4-7, ln(1/2))
                exh = [
                    expp.tile([P, 2, 512], F8, tag="exq", bufs=12, name=f"y{nm}_{i}")
                    for i in range(4)
                ]
                for tt in range(NT):
                    ps = psA.tile([P, 1024], F32, tag="ps_big", name=f"p1{nm}_{tt}")
                    dr_group(ps[:, 0:512], lhs, tt, rt, 512)
                    slot = slot0 if tt < 4 else slot0 + 1
                    nc.scalar.activation(
                        exh[tt // 2][:, tt % 2, :], ps[:, 0:512], AF.Exp,
                        scale=SC, bias=(0.0 if tt < 4 else lnhalf[:]),
                        accum_out=Dp[:, tt, slot : slot + 1],
                    )
                colsum(j, rs_slot, 0, [(exq[0], None), (exq[1], None)], f"a{nm}")
                colsum(j, rs_slot, 1, [(t, None) for t in exh], f"b{nm}")

            # S11 (needs AG1 only; overlaps AG2)
            for j in range(NCORES):
                sym_strip(n1_f8, ag1_out, j, D1p, 2 * j, 0, f"s11_{j}")
            # S22 (needs AG2)
            for j in range(NCORES):
                sym_strip(n2_f8, ag2_out, j, D2p, 2 * j, 1, f"s22_{j}")
            # S12 full strips (lhs n1, rhs gathered n2)
            for j in range(NCORES):
                rt = rhs_load(ag2_out, j, f"s12_{j}")
                exf = [
                    expp.tile([P, 2, 1024], F8, tag="exf", bufs=8, name=f"xf{j}_{i}")
                    for i in range(4)
                ]
                for tt in range(NT):
                    ps = psA.tile([P, 1024], F32, tag="ps_big", name=f"pf{j}_{tt}")
                    for ch in range(2):
                        dr_group(ps[:, bass.ts(ch, 512)], n1_f8, tt, rt, ch * 512)
                    if tt == 0:
                        flush_pending()
                    nc.scalar.activation(
                        exf[tt // 2][:, tt % 2, :], ps[:], AF.Exp, scale=SC,
                        accum_out=D1p[:, tt, 2 * NCORES + j : 2 * NCORES + j + 1],
                    )
                for h in range(2):
                    colsum(
                        j, 2, h,
                        [(t, bass.ds(h * 512, 512)) for t in exf],
                        f"f{j}_{h}",
                    )
            flush_pending()

            nc.gpsimd.collective_compute(
                "ReduceScatter", ALU.add, replica_groups=rg,
                ins=[rs_in[:].opt()], outs=[rs_out[:].opt()],
            )

            # ---------------- final loss ----------------
            r1s = small.tile([P, NT], F32, tag="r1s")
            r2s = small.tile([P, NT], F32, tag="r2s")
            nc.vector.reduce_sum(r1s[:], D1p[:], axis=mybir.AxisListType.X)
            nc.vector.reduce_sum(r2s[:], D2p[:], axis=mybir.AxisListType.X)
            c1 = small.tile([P, NT], F32, tag="c1")
            c2a = small.tile([P, NT], F32, tag="c2a")
            c2b = small.tile([P, NT], F32, tag="c2b")
            nc.sync.dma_start(c1[:], pt(rs_out[0]))
            nc.sync.dma_start(c2a[:], pt(rs_out[1]))
            nc.sync.dma_start(c2b[:], pt(rs_out[2]))
            p2 = small.tile([P, NT], F32, tag="p2")
            nc.sync.dma_start(p2[:], pt(p_dram[:]))

            d1 = small.tile([P, NT], F32, tag="d1")
            d2 = small.tile([P, NT], F32, tag="d2")
            nc.vector.tensor_tensor(d1[:], r1s[:], c1[:], ALU.add)
            nc.vector.tensor_scalar(d1[:], d1[:], -E2, None, ALU.add)
            nc.vector.tensor_tensor(d2[:], r2s[:], c2a[:], ALU.add)
            nc.vector.tensor_tensor(d2[:], d2[:], c2b[:], ALU.add)
            nc.vector.tensor_scalar(d2[:], d2[:], -E2, None, ALU.add)
            l1 = small.tile([P, NT], F32, tag="l1")
            l2 = small.tile([P, NT], F32, tag="l2")
            nc.scalar.activation(l1[:], d1[:], AF.Ln)
            nc.scalar.activation(l2[:], d2[:], AF.Ln)
            loss = small.tile([P, NT], F32, tag="loss")
            nc.vector.tensor_tensor(loss[:], l1[:], l2[:], ALU.add)
            nc.vector.tensor_scalar(loss[:], loss[:], 0.5, None, ALU.mult)
            pm = small.tile([P, NT], F32, tag="pm")
            nc.vector.tensor_scalar(pm[:], p2[:], -2.0, None, ALU.mult)
            nc.vector.tensor_tensor(loss[:], loss[:], pm[:], ALU.add)
            nc.sync.dma_start(pt(out[:]), loss[:])

    nc.finalize()
    return nc


@lru_cache(maxsize=1)
def _built():
    return _build()


def _prep_inputs(z1, z2, fc1_w, fc1_b, fc2_w, fc2_b):
    f8 = ml_dtypes.float8_e4m3
    w1 = np.asarray(fc1_w, np.float32)
    w2 = np.asarray(fc2_w, np.float32)
    w1t = np.ascontiguousarray(w1.T).astype(f8)
    w2t = np.ascontiguousarray(w2.T).astype(f8)
    b1 = np.asarray(fc1_b, np.float32)
    # device computes (elu+1) @ W2.T; correct with the fp8-rounded W2 row sums
    b2p = (
        np.asarray(fc2_b, np.float32) - w2.astype(f8).astype(np.float32).sum(axis=1)
    ).astype(np.float32)
    in_maps = []
    for c in range(NCORES):
        sl = slice(c * BLK, (c + 1) * BLK)
        in_maps.append(
            {
                "z1t": np.ascontiguousarray(np.asarray(z1[sl], np.float32).T).astype(f8),
                "z2t": np.ascontiguousarray(np.asarray(z2[sl], np.float32).T).astype(f8),
                "w1t": w1t,
                "w2t": w2t,
                "b1": b1,
                "b2p": b2p,
            }
        )
    return in_maps


def _install_ntff_shim():
    """Register the axon NTFF profile hook (antenv.axon_hooks is absent in
    this image; rebuild it from trn_agent_boot's ctypes recipe)."""
    import sys
    import types

    if "antenv.axon_hooks" in sys.modules:
        return True
    try:
        import antenv
        from trn_agent_boot.trn_boot import _ntff_profile_via_ctypes

        hook = _ntff_profile_via_ctypes("/opt/axon/libaxon_pjrt.so")
        if hook is None:
            return False
        m = types.ModuleType("antenv.axon_hooks")
        m._hook = hook
        m.get_axon_ntff_profile_hook = lambda: m._hook
        m.set_axon_ntff_profile_hook = lambda h: setattr(m, "_hook", h)
        sys.modules["antenv.axon_hooks"] = m
        antenv.axon_hooks = m
        # artifact upload needs egress; neuter it for local profiling
        import concourse.bass_utils as _bu

        _bu.upload_artifacts = lambda tmpdir: f"file://{tmpdir}"
        return True
    except Exception as e:
        print(f"ntff shim unavailable: {e!r}")
        return False


def _run(in_maps, trace=False):
    nc = _built()
    if trace and not _install_ntff_shim():
        trace = False
    last = None
    for attempt in range(3):
        try:
            res = run_bass_kernel_spmd(nc, in_maps, list(range(NCORES)), trace=trace)
            if all(np.isfinite(res.results[c]["out"]).all() for c in range(NCORES)):
                return res
            print("nonfinite output, retrying")
        except Exception as e:  # device occasionally wedged from a prior process
            last = e
            if "UNRECOVERABLE" not in str(e) and "UNAVAILABLE" not in str(e):
                raise
            print(f"device error (attempt {attempt}): retrying")
    if last is not None:
        raise last
    return res


def kernel(z1, z2, fc1_w, fc1_b, fc2_w, fc2_b):
    in_maps = _prep_inputs(z1, z2, fc1_w, fc1_b, fc2_w, fc2_b)
    res = _run(in_maps, trace=os.environ.get("KERNEL_TRACE", "") == "1")
    if res.exec_time_ns is not None:
        print(f"HW exec time: {res.exec_time_ns} ns")
    out = np.concatenate([res.results[c]["out"] for c in range(NCORES)])
    return out.astype(np.float32)
